# revision 5
# baseline (speedup 1.0000x reference)
"""Multi-head self-attention (B=2, S=4096, D=512, H=8, Dk=64) on 8 TRN2 cores.

Sharding: data-parallel over batch x head-parallel. Core c handles batch
c//4 and head pair (2*(c%4), 2*(c%4)+1). Each core computes Q/K/V
projections for its 128 model dims, full attention for its two heads, and
a partial output projection against its 128 rows of Wo. The host sums the
four partial outputs per batch and adds bo.

On-core layout (everything f32; matmuls run as float32r, 1 cyc/row):
  xT   [d, s]   via PE transposes of x tiles          (rhs for Q/K, lhsT for V)
  QT/KT [128, S] head0 in partitions 0:64, head1 in 64:128
  V    [s, 128] head0 in cols 0:64, head1 in 64:128   (lhsT for ctx)
  scoresT[k, q] blocks from row-packed matmul pairs (K=64 per head)
  attnT = exp(scoresT/8 + mask_bias) via one ACT op per [128, 1024] block
  ctxT [d, q] accumulated col-packed over k blocks; denominators from
  ones-vector matmuls into psum rows 0/32; normalization via a PE
  broadcast matmul of the reciprocals.
"""

import numpy as np
from contextlib import ExitStack

import concourse.bass as bass
import concourse.tile as tile
from concourse import bacc, mybir
from concourse.bass_utils import run_bass_kernel_spmd
from concourse.masks import make_identity

F32 = mybir.dt.float32
F32R = mybir.dt.float32r
BF16 = mybir.dt.bfloat16
EXP = mybir.ActivationFunctionType.Exp

D_MODEL = 512
N_HEADS = 8
D_K = 64
N_CORES = 8
DL = 128          # local model dims per core (2 heads)
Q_BLK = 512       # query block (free dim of scores matmuls)
SCALE = 1.0 / np.sqrt(D_K).item()


def _r(ap):
    return ap  # operand tensors are declared float32r


def build_kernel(ctx, tc, S, use_mask, use_bq, use_bk, use_bv, d):
    nc = tc.nc
    SB = S // 128    # s blocks of 128
    QB = S // Q_BLK  # query blocks of 512
    KB = S // 128    # key blocks of 128

    sp = ctx.enter_context(tc.tile_pool(name="sp", bufs=1))
    psum = ctx.enter_context(tc.tile_pool(name="psum", bufs=1, space="PSUM"))

    # ---- constants ----
    ident = sp.tile([128, 128], F32, tag="ident")
    make_identity(nc, ident)
    ones_f = sp.tile([128, 1], F32, tag="ones_f")
    nc.vector.memset(ones_f, 1.0)
    ones_col = sp.tile([128, 1], BF16, tag="ones_col")   # lhsT of denominator mms
    nc.vector.tensor_copy(ones_col, ones_f)
    ones_rep = sp.tile([33, 128], F32, tag="ones_rep")   # lhsT of broadcast mms
    nc.vector.memset(ones_rep, 1.0)

    # ---- weights ----
    wq_sb = sp.tile([128, 4, 128], F32R, tag="wq")
    nc.sync.dma_start(wq_sb, d["wq"].ap().rearrange("(t p) d -> p t d", p=128))
    wk_sb = sp.tile([128, 4, 128], F32R, tag="wk")
    nc.sync.dma_start(wk_sb, d["wk"].ap().rearrange("(t p) d -> p t d", p=128))
    wv_sb = sp.tile([128, 4, 128], F32R, tag="wv")
    nc.sync.dma_start(wv_sb, d["wv"].ap().rearrange("(t p) d -> p t d", p=128))
    wo_sb = sp.tile([128, 512], F32R, tag="wo")
    nc.sync.dma_start(wo_sb, d["wo"].ap())
    if use_bq:
        bq_sb = sp.tile([128, 1], F32, tag="bq")
        nc.sync.dma_start(bq_sb, d["bq"].ap()[:, None])
    if use_bk:
        bk_sb = sp.tile([128, 1], F32, tag="bk")
        nc.sync.dma_start(bk_sb, d["bk"].ap()[:, None])
    if use_bv:
        bv_sb = sp.tile([1, 128], F32, tag="bv")
        nc.sync.dma_start(bv_sb, d["bv"].ap()[None, :])
        ones_row = sp.tile([1, 128], F32, tag="ones_row")
        nc.vector.memset(ones_row, 1.0)
    if use_mask:
        mb_sb = sp.tile([128, KB], F32, tag="mb")
        nc.sync.dma_start(mb_sb, d["mb"].ap())

    # ---- phase 1: load x and transpose into xT [128, 4, S] ----
    xt = sp.tile([128, 4, S], F32R, tag="xt")
    for sb in range(SB):
        xs = sp.tile([128, 512], F32, tag="xs", bufs=3)
        nc.sync.dma_start(xs, d["x"].ap()[sb * 128:(sb + 1) * 128, :])
        for t in range(4):
            pt = psum.tile([128, 128], F32, tag="mm", bufs=2)
            nc.tensor.transpose(pt, xs[:, t * 128:(t + 1) * 128], ident)
            nc.vector.tensor_copy(xt[:, t, sb * 128:(sb + 1) * 128], pt)

    # ---- phase 2: projections ----
    qt = sp.tile([128, S], F32R, tag="qt")
    kt = sp.tile([128, S], F32R, tag="kt")
    for dst, w_sb, b_sb in (
        (qt, wq_sb, bq_sb if use_bq else None),
        (kt, wk_sb, bk_sb if use_bk else None),
    ):
        for sc in range(S // 512):
            pp = psum.tile([128, 512], F32, tag="mm", bufs=2)
            for t in range(4):
                nc.tensor.matmul(
                    pp, _r(w_sb[:, t, :]), _r(xt[:, t, sc * 512:(sc + 1) * 512]),
                    start=(t == 0), stop=(t == 3))
            out = dst[:, sc * 512:(sc + 1) * 512]
            if b_sb is not None:
                nc.vector.tensor_scalar_add(out, pp, b_sb[:, 0:1])
            else:
                nc.vector.tensor_copy(out, pp)

    v_all = sp.tile([128, SB, 128], BF16, tag="v")
    for sb in range(SB):
        pv = psum.tile([128, 128], F32, tag="mm", bufs=2)
        for t in range(4):
            nc.tensor.matmul(
                pv, _r(xt[:, t, sb * 128:(sb + 1) * 128]), _r(wv_sb[:, t, :]),
                start=(t == 0), stop=(t == 3 and not use_bv))
        if use_bv:
            nc.tensor.matmul(pv, ones_row[0:1, :], bv_sb[0:1, :],
                             start=False, stop=True)
        nc.vector.tensor_copy(v_all[:, sb, :], pv)

    # ---- phase 3: attention ----
    ctxn = sp.tile([128, S], F32R, tag="ctxn")
    for qb in range(QB):
        qs = slice(qb * Q_BLK, (qb + 1) * Q_BLK)
        pc = psum.tile([128, 512], F32, tag="ctx", bufs=1)
        pd = psum.tile([33, 512], F32, tag="den", bufs=1)
        for kb in range(KB):
            ks = slice(kb * 128, (kb + 1) * 128)
            ps = psum.tile([128, 1024], F32, tag="scores", bufs=2)
            nc.tensor.matmul(ps[:, 0:512], _r(kt[0:64, ks]), _r(qt[0:64, qs]))
            nc.tensor.matmul(ps[:, 512:1024], _r(kt[64:128, ks]), _r(qt[64:128, qs]))
            attn = sp.tile([128, 1024], BF16, tag="attn", bufs=3)
            nc.scalar.activation(
                attn, ps, EXP, scale=SCALE,
                bias=mb_sb[:, kb:kb + 1] if use_mask else 0.0)
            first, last = kb == 0, kb == KB - 1
            nc.tensor.matmul(pc[0:64, :], _r(v_all[:, kb, 0:64]),
                             _r(attn[:, 0:512]), start=first, stop=last,
                             skip_group_check=True)
            nc.tensor.matmul(pc[64:128, :], _r(v_all[:, kb, 64:128]),
                             _r(attn[:, 512:1024]), start=first, stop=last,
                             skip_group_check=True)
            nc.tensor.matmul(pd[0:1, :], _r(ones_col[:, 0:1]),
                             _r(attn[:, 0:512]), start=first, stop=last,
                             skip_group_check=True)
            nc.tensor.matmul(pd[32:33, :], _r(ones_col[:, 0:1]),
                             _r(attn[:, 512:1024]), start=first, stop=last,
                             skip_group_check=True)

        rcp = sp.tile([33, 512], F32, tag="rcp", bufs=2)
        nc.vector.reciprocal(rcp[0:1, :], pd[0:1, :])
        nc.vector.reciprocal(rcp[32:33, :], pd[32:33, :])
        pr0 = psum.tile([128, 512], F32, tag="mm", bufs=2)
        nc.tensor.matmul(pr0, ones_rep[0:1, :], rcp[0:1, :])
        rep0 = sp.tile([128, 512], F32, tag="rep", bufs=2)
        nc.vector.tensor_copy(rep0, pr0)
        pr1 = psum.tile([128, 512], F32, tag="mm", bufs=2)
        nc.tensor.matmul(pr1, ones_rep[32:33, :], rcp[32:33, :])
        rep1 = sp.tile([128, 512], F32, tag="rep", bufs=2)
        nc.vector.tensor_copy(rep1, pr1)
        nc.vector.tensor_mul(ctxn[0:64, qs], pc[0:64, :], rep0[0:64, :])
        nc.vector.tensor_mul(ctxn[64:128, qs], pc[64:128, :], rep1[64:128, :])

        # ---- phase 4: output projection for this query block ----
        for i in range(Q_BLK // 128):
            sb = qb * (Q_BLK // 128) + i
            po = psum.tile([128, 512], F32, tag="mm", bufs=2)
            nc.tensor.matmul(po, _r(ctxn[:, sb * 128:(sb + 1) * 128]), _r(wo_sb))
            ob = sp.tile([128, 512], F32, tag="ob", bufs=3)
            nc.vector.tensor_copy(ob, po)
            nc.sync.dma_start(d["out"].ap()[sb * 128:(sb + 1) * 128, :], ob)


def build_program(S=4096, use_mask=False, use_bq=False, use_bk=False,
                  use_bv=False, enable_asserts=False):
    nc = bacc.Bacc("TRN2", target_bir_lowering=False, debug=False,
                   enable_asserts=enable_asserts, num_devices=N_CORES,
                   name="mha")
    d = {
        "x": nc.dram_tensor("x", [S, D_MODEL], F32, kind="ExternalInput"),
        "wq": nc.dram_tensor("wq", [D_MODEL, DL], F32R, kind="ExternalInput"),
        "wk": nc.dram_tensor("wk", [D_MODEL, DL], F32R, kind="ExternalInput"),
        "wv": nc.dram_tensor("wv", [D_MODEL, DL], F32R, kind="ExternalInput"),
        "wo": nc.dram_tensor("wo", [DL, D_MODEL], F32R, kind="ExternalInput"),
        "out": nc.dram_tensor("out", [S, D_MODEL], F32, kind="ExternalOutput"),
    }
    if use_bq:
        d["bq"] = nc.dram_tensor("bq", [DL], F32, kind="ExternalInput")
    if use_bk:
        d["bk"] = nc.dram_tensor("bk", [DL], F32, kind="ExternalInput")
    if use_bv:
        d["bv"] = nc.dram_tensor("bv", [DL], F32, kind="ExternalInput")
    if use_mask:
        d["mb"] = nc.dram_tensor("mb", [128, S // 128], F32,
                                 kind="ExternalInput")
    with tile.TileContext(nc) as tc:
        with ExitStack() as ctx:
            build_kernel(ctx, tc, S, use_mask, use_bq, use_bk, use_bv, d)
    nc.compile()
    return nc


_cache = {}


def _program(key):
    if key not in _cache:
        _cache[key] = build_program(
            S=4096, use_mask=key[0], use_bq=key[1], use_bk=key[2],
            use_bv=key[3])
    return _cache[key]


def kernel(x, mask, Wq, bq, Wk, bk, Wv, bv, Wo, bo, _results_hook=None):
    x = np.asarray(x, np.float32)
    mask = np.asarray(mask)
    B, S, _ = x.shape
    use_mask = bool((mask == 0).any())
    use_bq = bool(np.asarray(bq).any())
    use_bk = bool(np.asarray(bk).any())
    use_bv = bool(np.asarray(bv).any())
    nc = _program((use_mask, use_bq, use_bk, use_bv))

    in_maps = []
    for c in range(N_CORES):
        b, j = divmod(c, N_CORES // B)
        ds = slice(j * DL, (j + 1) * DL)
        m = {
            "x": np.ascontiguousarray(x[b]),
            "wq": np.ascontiguousarray(Wq[:, ds], dtype=np.float32),
            "wk": np.ascontiguousarray(Wk[:, ds], dtype=np.float32),
            "wv": np.ascontiguousarray(Wv[:, ds], dtype=np.float32),
            "wo": np.ascontiguousarray(Wo[ds, :], dtype=np.float32),
        }
        if use_bq:
            m["bq"] = np.ascontiguousarray(bq[ds], dtype=np.float32)
        if use_bk:
            m["bk"] = np.ascontiguousarray(bk[ds], dtype=np.float32)
        if use_bv:
            m["bv"] = np.ascontiguousarray(bv[ds], dtype=np.float32)
        if use_mask:
            mb = np.where(np.asarray(mask[b]) == 0, -1e9, 0.0).astype(np.float32)
            m["mb"] = np.ascontiguousarray(mb.reshape(S // 128, 128).T)
        in_maps.append(m)

    res = run_bass_kernel_spmd(nc, in_maps, core_ids=list(range(N_CORES)))
    if _results_hook is not None:
        _results_hook(res)
    out = np.zeros((B, S, D_MODEL), np.float32)
    for c in range(N_CORES):
        b = c // (N_CORES // B)
        out[b] += res.results[c]["out"]
    out += np.asarray(bo, np.float32)
    return out


# revision 9
# speedup vs baseline: 1.0691x; 1.0691x over previous
"""Multi-head self-attention (B=2, S=4096, D=512, H=8, Dk=64) on 8 TRN2 cores.

Sharding: data-parallel over batch x head-parallel. Core c handles batch
c//4 and head pair (2*(c%4), 2*(c%4)+1). Each core computes Q/K/V
projections for its 128 model dims, full attention for its two heads, and
a partial output projection against its 128 rows of Wo. The host sums the
four partial outputs per batch and adds bo.

On-core layout (bf16 operands, fp32 psum accumulation):
  xT   [d, s]   bf16 via DMA-transpose of host-cast x     (rhs for Q/K, lhsT for V)
  QT/KT [128, S] bf16, head0 in partitions 0:64, head1 in 64:128
  V    [s, 128] bf16, head0 in cols 0:64, head1 in 64:128 (lhsT for ctx)
  scoresT[k, q] fp32 psum from row-packed bf16 matmul pairs (K=64/head)
  attnT = exp(scoresT/8 + mask_bias) bf16, one ACT op per [128, 1024] block
  ctxT [d, q] fp32 psum, col-packed over k blocks; denominators from
  ones-vector matmuls into psum rows 0/32; normalization via fp32 PE
  broadcast of the reciprocals.
"""

import numpy as np
import ml_dtypes
from contextlib import ExitStack

import concourse.bass as bass
import concourse.tile as tile
from concourse import bacc, mybir
from concourse.bass_utils import run_bass_kernel_spmd
from concourse.tile_rust import add_dep_helper

F32 = mybir.dt.float32
F32R = mybir.dt.float32r
BF16 = mybir.dt.bfloat16
EXP = mybir.ActivationFunctionType.Exp

D_MODEL = 512
N_HEADS = 8
D_K = 64
N_CORES = 8
DL = 128          # local model dims per core (2 heads)
Q_BLK = 512       # query block (free dim of scores matmuls)
SCALE = 1.0 / np.sqrt(D_K).item()


def build_kernel(ctx, tc, S, use_mask, use_bq, use_bk, use_bv, d):
    nc = tc.nc
    SB = S // 128    # s blocks of 128
    QB = S // Q_BLK  # query blocks of 512
    KB = S // 128    # key blocks of 128

    sp = ctx.enter_context(tc.tile_pool(name="sp", bufs=1))
    psum = ctx.enter_context(tc.tile_pool(name="psum", bufs=1, space="PSUM"))
    # psum budget (8 banks): scores 2x[128,1024]=4, ctx 2x[128,512]=2,
    # den 2x[<=1 bank]=2. All other matmul outputs share the ctx/den tags.

    # ---- constants ----
    ones_f = sp.tile([128, 1], F32, tag="ones_f")
    nc.vector.memset(ones_f, 1.0)
    ones_col = sp.tile([128, 1], BF16, tag="ones_col")  # lhsT of denominator mms
    nc.vector.tensor_copy(ones_col, ones_f)
    ones_rep = sp.tile([33, 128], F32, tag="ones_rep")  # lhsT of broadcast mms
    nc.vector.memset(ones_rep, 1.0)

    # ---- phase 1: DMA-transpose x (bf16) into xT [128, 4, S] ----
    # The xbar-transpose DMAs must not overlap regular DMAs (S2M xbar-mode
    # transition hazard), so they go first and every other DMA waits on them.
    xt = sp.tile([128, 4, S], BF16, tag="xt")
    t_insts = []
    for t in range(4):
        t_insts.append(nc.sync.dma_start_transpose(
            xt[:, t, :], d["xb"].ap()[:, t * 128:(t + 1) * 128]))

    def dma_after_transpose(out_ap, in_ap):
        ins = nc.sync.dma_start(out_ap, in_ap)
        for t_inst in t_insts:
            add_dep_helper(ins.ins, t_inst.ins, reason="xbar-mode serialize")
        return ins

    # ---- weights ----
    wq_sb = sp.tile([128, 4, 128], BF16, tag="wq")
    dma_after_transpose(wq_sb, d["wq"].ap().rearrange("(t p) d -> p t d", p=128))
    wk_sb = sp.tile([128, 4, 128], BF16, tag="wk")
    dma_after_transpose(wk_sb, d["wk"].ap().rearrange("(t p) d -> p t d", p=128))
    wv_sb = sp.tile([128, 4, 128], BF16, tag="wv")
    dma_after_transpose(wv_sb, d["wv"].ap().rearrange("(t p) d -> p t d", p=128))
    wo_sb = sp.tile([128, 512], F32R, tag="wo")
    dma_after_transpose(wo_sb, d["wo"].ap())
    if use_bq:
        bq_sb = sp.tile([128, 1], F32, tag="bq")
        dma_after_transpose(bq_sb, d["bq"].ap()[:, None])
    if use_bk:
        bk_sb = sp.tile([128, 1], F32, tag="bk")
        dma_after_transpose(bk_sb, d["bk"].ap()[:, None])
    if use_bv:
        bv_sb = sp.tile([1, 128], F32, tag="bv")
        dma_after_transpose(bv_sb, d["bv"].ap()[None, :])
        ones_row = sp.tile([1, 128], F32, tag="ones_row")
        nc.vector.memset(ones_row, 1.0)
    if use_mask:
        mb_sb = sp.tile([128, KB], F32, tag="mb")
        dma_after_transpose(mb_sb, d["mb"].ap())

    # ---- phase 2: projections ----
    qt = sp.tile([128, S], BF16, tag="qt")
    kt = sp.tile([128, S], BF16, tag="kt")
    for dst, w_sb, b_sb in (
        (qt, wq_sb, bq_sb if use_bq else None),
        (kt, wk_sb, bk_sb if use_bk else None),
    ):
        for sc in range(S // 512):
            pp = psum.tile([128, 512], F32, tag="ctx", bufs=2)
            for t in range(4):
                nc.tensor.matmul(
                    pp, w_sb[:, t, :], xt[:, t, sc * 512:(sc + 1) * 512],
                    start=(t == 0), stop=(t == 3))
            out = dst[:, sc * 512:(sc + 1) * 512]
            if b_sb is not None:
                nc.vector.tensor_scalar_add(out, pp, b_sb[:, 0:1])
            else:
                nc.vector.tensor_copy(out, pp)

    v_all = sp.tile([128, SB, 128], BF16, tag="v")
    for sb in range(SB):
        pv = psum.tile([128, 128], F32, tag="den", bufs=2)
        for t in range(4):
            nc.tensor.matmul(
                pv, xt[:, t, sb * 128:(sb + 1) * 128], wv_sb[:, t, :],
                start=(t == 0), stop=(t == 3 and not use_bv))
        if use_bv:
            nc.tensor.matmul(pv, ones_row[0:1, :], bv_sb[0:1, :],
                             start=False, stop=True)
        nc.vector.tensor_copy(v_all[:, sb, :], pv)

    if "dbg_qt" in d:
        nc.sync.dma_start(d["dbg_qt"].ap(), qt)
        nc.sync.dma_start(d["dbg_kt"].ap(), kt)
        dbg_v = sp.tile([128, SB, 128], BF16, tag="dbg_v")
        nc.vector.tensor_copy(dbg_v, v_all)
        nc.sync.dma_start(d["dbg_v"].ap().rearrange("(n p) d -> p n d", p=128), dbg_v)
        dbg_xt = sp.tile([128, 4, S], BF16, tag="dbg_xt")
        nc.vector.tensor_copy(dbg_xt, xt)
        nc.sync.dma_start(d["dbg_xt"].ap().rearrange("(t p) s -> p t s", p=128), dbg_xt)

    # ---- phase 3: attention ----
    ctxn = sp.tile([128, S], F32R, tag="ctxn")
    for qb in range(QB):
        qs = slice(qb * Q_BLK, (qb + 1) * Q_BLK)
        pc = psum.tile([128, 512], F32, tag="ctx", bufs=2)
        pd = psum.tile([33, 512], F32, tag="den", bufs=2)
        for kb in range(KB):
            ks = slice(kb * 128, (kb + 1) * 128)
            ps = psum.tile([128, 1024], F32, tag="scores", bufs=2)
            nc.tensor.matmul(ps[:, 0:512], kt[0:64, ks], qt[0:64, qs])
            nc.tensor.matmul(ps[:, 512:1024], kt[64:128, ks], qt[64:128, qs])
            attn = sp.tile([128, 1024], BF16, tag="attn", bufs=3)
            nc.scalar.activation(
                attn, ps, EXP, scale=SCALE,
                bias=mb_sb[:, kb:kb + 1] if use_mask else 0.0)
            first, last = kb == 0, kb == KB - 1
            nc.tensor.matmul(pc[0:64, :], v_all[:, kb, 0:64],
                             attn[:, 0:512], start=first, stop=last,
                             skip_group_check=True)
            nc.tensor.matmul(pc[64:128, :], v_all[:, kb, 64:128],
                             attn[:, 512:1024], start=first, stop=last,
                             skip_group_check=True)
            nc.tensor.matmul(pd[0:1, :], ones_col[:, 0:1],
                             attn[:, 0:512], start=first, stop=last,
                             skip_group_check=True)
            nc.tensor.matmul(pd[32:33, :], ones_col[:, 0:1],
                             attn[:, 512:1024], start=first, stop=last,
                             skip_group_check=True)

        # copy denominators out of psum quickly (frees the den slot), then
        # reciprocal + PE broadcast + normalize — all off the critical path.
        den_sb = sp.tile([33, 512], F32, tag="den_sb", bufs=2)
        nc.vector.tensor_copy(den_sb[0:1, :], pd[0:1, :])
        nc.vector.tensor_copy(den_sb[32:33, :], pd[32:33, :])
        rcp = sp.tile([33, 512], F32, tag="rcp", bufs=2)
        nc.vector.reciprocal(rcp[0:1, :], den_sb[0:1, :])
        nc.vector.reciprocal(rcp[32:33, :], den_sb[32:33, :])
        pr0 = psum.tile([128, 512], F32, tag="den", bufs=2)
        nc.tensor.matmul(pr0, ones_rep[0:1, :], rcp[0:1, :])
        rep0 = sp.tile([128, 512], F32, tag="rep", bufs=2)
        nc.vector.tensor_copy(rep0, pr0)
        pr1 = psum.tile([128, 512], F32, tag="den", bufs=2)
        nc.tensor.matmul(pr1, ones_rep[32:33, :], rcp[32:33, :])
        rep1 = sp.tile([128, 512], F32, tag="rep", bufs=2)
        nc.vector.tensor_copy(rep1, pr1)
        nc.vector.tensor_mul(ctxn[0:64, qs], pc[0:64, :], rep0[0:64, :])
        nc.vector.tensor_mul(ctxn[64:128, qs], pc[64:128, :], rep1[64:128, :])
        if qb == 0 and "dbg_den" in d:
            nc.sync.dma_start(d["dbg_den"].ap(), den_sb)
            nc.sync.dma_start(d["dbg_rcp"].ap(), rcp)
            dbg_rep = sp.tile([128, 1024], F32, tag="dbg_rep")
            nc.vector.tensor_copy(dbg_rep[:, 0:512], rep0)
            nc.vector.tensor_copy(dbg_rep[:, 512:1024], rep1)
            nc.sync.dma_start(d["dbg_rep"].ap(), dbg_rep)
            dbg_pc = sp.tile([128, 512], F32, tag="dbg_pc")
            nc.vector.tensor_copy(dbg_pc, pc)
            nc.sync.dma_start(d["dbg_pc"].ap(), dbg_pc)

        # ---- phase 4: output projection for this query block ----
        for i in range(Q_BLK // 128):
            sb = qb * (Q_BLK // 128) + i
            po = psum.tile([128, 512], F32, tag="ctx", bufs=2)
            nc.tensor.matmul(po, ctxn[:, sb * 128:(sb + 1) * 128], wo_sb)
            ob = sp.tile([128, 512], F32, tag="ob", bufs=3)
            nc.vector.tensor_copy(ob, po)
            nc.sync.dma_start(d["out"].ap()[sb * 128:(sb + 1) * 128, :], ob)


def build_program(S=4096, use_mask=False, use_bq=False, use_bk=False,
                  use_bv=False, enable_asserts=False):
    nc = bacc.Bacc("TRN2", target_bir_lowering=False, debug=False,
                   enable_asserts=enable_asserts, num_devices=N_CORES,
                   name="mha")
    d = {
        "xb": nc.dram_tensor("xb", [S, D_MODEL], BF16, kind="ExternalInput"),
        "wq": nc.dram_tensor("wq", [D_MODEL, DL], BF16, kind="ExternalInput"),
        "wk": nc.dram_tensor("wk", [D_MODEL, DL], BF16, kind="ExternalInput"),
        "wv": nc.dram_tensor("wv", [D_MODEL, DL], BF16, kind="ExternalInput"),
        "wo": nc.dram_tensor("wo", [DL, D_MODEL], F32R, kind="ExternalInput"),
        "out": nc.dram_tensor("out", [S, D_MODEL], F32, kind="ExternalOutput"),
    }
    if use_bq:
        d["bq"] = nc.dram_tensor("bq", [DL], F32, kind="ExternalInput")
    if use_bk:
        d["bk"] = nc.dram_tensor("bk", [DL], F32, kind="ExternalInput")
    if use_bv:
        d["bv"] = nc.dram_tensor("bv", [DL], F32, kind="ExternalInput")
    if use_mask:
        d["mb"] = nc.dram_tensor("mb", [128, S // 128], F32,
                                 kind="ExternalInput")
    if S <= 1024:
        import os
        if os.environ.get("MHA_DEBUG"):
            d["dbg_den"] = nc.dram_tensor("dbg_den", [33, 512], F32, kind="ExternalOutput")
            d["dbg_rcp"] = nc.dram_tensor("dbg_rcp", [33, 512], F32, kind="ExternalOutput")
            d["dbg_rep"] = nc.dram_tensor("dbg_rep", [128, 1024], F32, kind="ExternalOutput")
            d["dbg_pc"] = nc.dram_tensor("dbg_pc", [128, 512], F32, kind="ExternalOutput")
            d["dbg_qt"] = nc.dram_tensor("dbg_qt", [128, S], BF16, kind="ExternalOutput")
            d["dbg_kt"] = nc.dram_tensor("dbg_kt", [128, S], BF16, kind="ExternalOutput")
            d["dbg_v"] = nc.dram_tensor("dbg_v", [S, 128], BF16, kind="ExternalOutput")
            d["dbg_xt"] = nc.dram_tensor("dbg_xt", [512, S], BF16, kind="ExternalOutput")
    with tile.TileContext(nc) as tc:
        with ExitStack() as ctx:
            build_kernel(ctx, tc, S, use_mask, use_bq, use_bk, use_bv, d)
    nc.compile()
    return nc


_cache = {}


def _program(key):
    if key not in _cache:
        _cache[key] = build_program(
            S=4096, use_mask=key[0], use_bq=key[1], use_bk=key[2],
            use_bv=key[3])
    return _cache[key]


def kernel(x, mask, Wq, bq, Wk, bk, Wv, bv, Wo, bo, _results_hook=None):
    x = np.asarray(x, np.float32)
    mask = np.asarray(mask)
    B, S, _ = x.shape
    use_mask = bool((mask == 0).any())
    use_bq = bool(np.asarray(bq).any())
    use_bk = bool(np.asarray(bk).any())
    use_bv = bool(np.asarray(bv).any())
    nc = _program((use_mask, use_bq, use_bk, use_bv))

    in_maps = []
    for c in range(N_CORES):
        b, j = divmod(c, N_CORES // B)
        ds = slice(j * DL, (j + 1) * DL)
        m = {
            "xb": np.ascontiguousarray(x[b]).astype(ml_dtypes.bfloat16),
            "wq": np.ascontiguousarray(Wq[:, ds]).astype(ml_dtypes.bfloat16),
            "wk": np.ascontiguousarray(Wk[:, ds]).astype(ml_dtypes.bfloat16),
            "wv": np.ascontiguousarray(Wv[:, ds]).astype(ml_dtypes.bfloat16),
            "wo": np.ascontiguousarray(Wo[ds, :], dtype=np.float32),
        }
        if use_bq:
            m["bq"] = np.ascontiguousarray(bq[ds], dtype=np.float32)
        if use_bk:
            m["bk"] = np.ascontiguousarray(bk[ds], dtype=np.float32)
        if use_bv:
            m["bv"] = np.ascontiguousarray(bv[ds], dtype=np.float32)
        if use_mask:
            mb = np.where(np.asarray(mask[b]) == 0, -1e9, 0.0).astype(np.float32)
            m["mb"] = np.ascontiguousarray(mb.reshape(S // 128, 128).T)
        in_maps.append(m)

    res = run_bass_kernel_spmd(nc, in_maps, core_ids=list(range(N_CORES)))
    if _results_hook is not None:
        _results_hook(res)
    out = np.zeros((B, S, D_MODEL), np.float32)
    for c in range(N_CORES):
        b = c // (N_CORES // B)
        out[b] += res.results[c]["out"]
    out += np.asarray(bo, np.float32)
    return out


# revision 11
# speedup vs baseline: 1.4114x; 1.3202x over previous
"""Multi-head self-attention (B=2, S=4096, D=512, H=8, Dk=64) on 8 TRN2 cores.

Sharding: data-parallel over batch x head-parallel. Core c handles batch
c//4 and head pair (2*(c%4), 2*(c%4)+1). Each core computes Q/K/V
projections for its 128 model dims, full attention for its two heads, and
a partial output projection against its 128 rows of Wo. The host sums the
four partial outputs per batch and adds bo.

On-core layout (bf16 operands, fp32 psum accumulation):
  xT   [d, s]   bf16 via DMA-transpose of host-cast x     (rhs for Q/K, lhsT for V)
  QT/KT [128, S] bf16, head0 in partitions 0:64, head1 in 64:128
  V    [s, 128] bf16, head0 in cols 0:64, head1 in 64:128 (lhsT for ctx)
  scoresT[k, q] fp32 psum from row-packed bf16 matmul pairs (K=64/head)
  attnT = exp(scoresT/8 + mask_bias) bf16, one ACT op per [128, 1024] block
  ctxT [d, q] fp32 psum, col-packed over k blocks; denominators from
  ones-vector matmuls into psum rows 0/32; normalization via fp32 PE
  broadcast of the reciprocals.
"""

import numpy as np
import ml_dtypes
from contextlib import ExitStack

import concourse.bass as bass
import concourse.tile as tile
from concourse import bacc, mybir
from concourse.bass_utils import run_bass_kernel_spmd
from concourse.tile_rust import add_dep_helper

F32 = mybir.dt.float32
F32R = mybir.dt.float32r
BF16 = mybir.dt.bfloat16
EXP = mybir.ActivationFunctionType.Exp

D_MODEL = 512
N_HEADS = 8
D_K = 64
N_CORES = 8
DL = 128          # local model dims per core (2 heads)
Q_BLK = 512       # query block (free dim of scores matmuls)
SCALE = 1.0 / np.sqrt(D_K).item()


def build_kernel(ctx, tc, S, use_mask, use_bq, use_bk, use_bv, d):
    nc = tc.nc
    SB = S // 128    # s blocks of 128
    QB = S // Q_BLK  # query blocks of 512
    KB = S // 128    # key blocks of 128

    sp = ctx.enter_context(tc.tile_pool(name="sp", bufs=1))
    psum = ctx.enter_context(tc.tile_pool(name="psum", bufs=1, space="PSUM"))
    # psum budget (8 banks): scores 2x[128,1024]=4, ctx 2x[128,512]=2,
    # den 2x[<=1 bank]=2. All other matmul outputs share the ctx/den tags.

    # ---- constants ----
    ones_f = sp.tile([128, 1], F32, tag="ones_f")
    nc.vector.memset(ones_f, 1.0)
    ones_col = sp.tile([128, 1], BF16, tag="ones_col")  # lhsT of denominator mms
    nc.vector.tensor_copy(ones_col, ones_f)
    ones_rep = sp.tile([33, 128], F32, tag="ones_rep")  # lhsT of broadcast mms
    nc.vector.memset(ones_rep, 1.0)

    # ---- phase 1: DMA-transpose x (bf16) into xT [128, 4, S] ----
    # The xbar-transpose DMAs must not overlap regular DMAs (S2M xbar-mode
    # transition hazard), so they go first and every other DMA waits on them.
    xt = sp.tile([128, 4, S], BF16, tag="xt")
    t_insts = []
    for t in range(4):
        t_insts.append(nc.sync.dma_start_transpose(
            xt[:, t, :], d["xb"].ap()[:, t * 128:(t + 1) * 128]))

    def dma_after_transpose(out_ap, in_ap):
        ins = nc.sync.dma_start(out_ap, in_ap)
        for t_inst in t_insts:
            add_dep_helper(ins.ins, t_inst.ins, reason="xbar-mode serialize")
        return ins

    # ---- weights ----
    wq_sb = sp.tile([128, 4, 128], BF16, tag="wq")
    dma_after_transpose(wq_sb, d["wq"].ap().rearrange("(t p) d -> p t d", p=128))
    wk_sb = sp.tile([128, 4, 128], BF16, tag="wk")
    dma_after_transpose(wk_sb, d["wk"].ap().rearrange("(t p) d -> p t d", p=128))
    wv_sb = sp.tile([128, 4, 128], BF16, tag="wv")
    dma_after_transpose(wv_sb, d["wv"].ap().rearrange("(t p) d -> p t d", p=128))
    wo_sb = sp.tile([128, 512], F32R, tag="wo")
    dma_after_transpose(wo_sb, d["wo"].ap())
    if use_bq:
        bq_sb = sp.tile([128, 1], F32, tag="bq")
        dma_after_transpose(bq_sb, d["bq"].ap()[:, None])
    if use_bk:
        bk_sb = sp.tile([128, 1], F32, tag="bk")
        dma_after_transpose(bk_sb, d["bk"].ap()[:, None])
    if use_bv:
        bv_sb = sp.tile([1, 128], F32, tag="bv")
        dma_after_transpose(bv_sb, d["bv"].ap()[None, :])
        ones_row = sp.tile([1, 128], F32, tag="ones_row")
        nc.vector.memset(ones_row, 1.0)
    if use_mask:
        mb_sb = sp.tile([128, KB], F32, tag="mb")
        dma_after_transpose(mb_sb, d["mb"].ap())

    # ---- phase 2: projections ----
    qt = sp.tile([128, S], BF16, tag="qt")
    kt = sp.tile([128, S], BF16, tag="kt")
    for dst, w_sb, b_sb in (
        (qt, wq_sb, bq_sb if use_bq else None),
        (kt, wk_sb, bk_sb if use_bk else None),
    ):
        for sc in range(S // 512):
            pp = psum.tile([128, 512], F32, tag="ctx", bufs=2)
            for t in range(4):
                nc.tensor.matmul(
                    pp, w_sb[:, t, :], xt[:, t, sc * 512:(sc + 1) * 512],
                    start=(t == 0), stop=(t == 3))
            out = dst[:, sc * 512:(sc + 1) * 512]
            if b_sb is not None:
                nc.vector.tensor_scalar_add(out, pp, b_sb[:, 0:1])
            else:
                nc.vector.tensor_copy(out, pp)

    v_all = sp.tile([128, SB, 128], BF16, tag="v")
    for sb in range(SB):
        pv = psum.tile([128, 128], F32, tag="den", bufs=2)
        for t in range(4):
            nc.tensor.matmul(
                pv, xt[:, t, sb * 128:(sb + 1) * 128], wv_sb[:, t, :],
                start=(t == 0), stop=(t == 3 and not use_bv))
        if use_bv:
            nc.tensor.matmul(pv, ones_row[0:1, :], bv_sb[0:1, :],
                             start=False, stop=True)
        nc.vector.tensor_copy(v_all[:, sb, :], pv)

    # ---- phase 3: attention ----
    ctxn = sp.tile([128, S], F32R, tag="ctxn")
    pending_tail = None
    for qb in range(QB):
        qs = slice(qb * Q_BLK, (qb + 1) * Q_BLK)
        pc = psum.tile([128, 512], F32, tag="ctx", bufs=2)
        pd = psum.tile([33, 512], F32, tag="den", bufs=2)
        for kb in range(KB):
            ks = slice(kb * 128, (kb + 1) * 128)
            ps = psum.tile([128, 1024], F32, tag="scores", bufs=2)
            nc.tensor.matmul(ps[:, 0:512], kt[0:64, ks], qt[0:64, qs])
            nc.tensor.matmul(ps[:, 512:1024], kt[64:128, ks], qt[64:128, qs])
            attn = sp.tile([128, 1024], BF16, tag="attn", bufs=3)
            nc.scalar.activation(
                attn, ps, EXP, scale=SCALE,
                bias=mb_sb[:, kb:kb + 1] if use_mask else 0.0)
            first, last = kb == 0, kb == KB - 1
            nc.tensor.matmul(pc[0:64, :], v_all[:, kb, 0:64],
                             attn[:, 0:512], start=first, stop=last,
                             skip_group_check=True)
            nc.tensor.matmul(pc[64:128, :], v_all[:, kb, 64:128],
                             attn[:, 512:1024], start=first, stop=last,
                             skip_group_check=True)
            nc.tensor.matmul(pd[0:1, :], ones_col[:, 0:1],
                             attn[:, 0:512], start=first, stop=last,
                             skip_group_check=True)
            nc.tensor.matmul(pd[32:33, :], ones_col[:, 0:1],
                             attn[:, 512:1024], start=first, stop=last,
                             skip_group_check=True)

        # Denominator extraction + reciprocals start immediately (DVE is
        # idle during the matmul loop)...
        den_sb = sp.tile([33, 512], F32, tag="den_sb", bufs=2)
        nc.vector.tensor_copy(den_sb[0:1, :], pd[0:1, :])
        nc.vector.tensor_copy(den_sb[32:33, :], pd[32:33, :])
        rcp = sp.tile([33, 512], F32, tag="rcp", bufs=2)
        nc.vector.reciprocal(rcp[0:1, :], den_sb[0:1, :])
        nc.vector.reciprocal(rcp[32:33, :], den_sb[32:33, :])

        # ...but the PE part of the tail (broadcast matmuls + output
        # projection) is deferred by one query block, so the PE never
        # stalls on the reciprocal chain — that stall re-throttles the
        # HAM clock to K=4/8 and halves matmul throughput.
        def tail(qb=qb, qs=qs, pc=pc, rcp=rcp):
            pr0 = psum.tile([128, 512], F32, tag="den", bufs=2, name="pr0")
            nc.tensor.matmul(pr0, ones_rep[0:1, :], rcp[0:1, :])
            rep0 = sp.tile([128, 512], F32, tag="rep", bufs=2, name="rep0")
            nc.vector.tensor_copy(rep0, pr0)
            pr1 = psum.tile([128, 512], F32, tag="den", bufs=2, name="pr1")
            nc.tensor.matmul(pr1, ones_rep[32:33, :], rcp[32:33, :])
            rep1 = sp.tile([128, 512], F32, tag="rep", bufs=2, name="rep1")
            nc.vector.tensor_copy(rep1, pr1)
            nc.vector.tensor_mul(ctxn[0:64, qs], pc[0:64, :], rep0[0:64, :])
            nc.vector.tensor_mul(ctxn[64:128, qs], pc[64:128, :], rep1[64:128, :])
            for i in range(Q_BLK // 128):
                sb = qb * (Q_BLK // 128) + i
                po = psum.tile([128, 512], F32, tag="ctx", bufs=2, name="po")
                nc.tensor.matmul(po, ctxn[:, sb * 128:(sb + 1) * 128], wo_sb)
                ob = sp.tile([128, 512], F32, tag="ob", bufs=3, name="ob")
                nc.vector.tensor_copy(ob, po)
                nc.sync.dma_start(d["out"].ap()[sb * 128:(sb + 1) * 128, :], ob)

        if pending_tail is not None:
            pending_tail()
        pending_tail = tail

    pending_tail()


def build_program(S=4096, use_mask=False, use_bq=False, use_bk=False,
                  use_bv=False, enable_asserts=False):
    nc = bacc.Bacc("TRN2", target_bir_lowering=False, debug=False,
                   enable_asserts=enable_asserts, num_devices=N_CORES,
                   name="mha")
    d = {
        "xb": nc.dram_tensor("xb", [S, D_MODEL], BF16, kind="ExternalInput"),
        "wq": nc.dram_tensor("wq", [D_MODEL, DL], BF16, kind="ExternalInput"),
        "wk": nc.dram_tensor("wk", [D_MODEL, DL], BF16, kind="ExternalInput"),
        "wv": nc.dram_tensor("wv", [D_MODEL, DL], BF16, kind="ExternalInput"),
        "wo": nc.dram_tensor("wo", [DL, D_MODEL], F32R, kind="ExternalInput"),
        "out": nc.dram_tensor("out", [S, D_MODEL], F32, kind="ExternalOutput"),
    }
    if use_bq:
        d["bq"] = nc.dram_tensor("bq", [DL], F32, kind="ExternalInput")
    if use_bk:
        d["bk"] = nc.dram_tensor("bk", [DL], F32, kind="ExternalInput")
    if use_bv:
        d["bv"] = nc.dram_tensor("bv", [DL], F32, kind="ExternalInput")
    if use_mask:
        d["mb"] = nc.dram_tensor("mb", [128, S // 128], F32,
                                 kind="ExternalInput")
    with tile.TileContext(nc) as tc:
        with ExitStack() as ctx:
            build_kernel(ctx, tc, S, use_mask, use_bq, use_bk, use_bv, d)
    nc.compile()
    return nc


_cache = {}


def _program(key):
    if key not in _cache:
        _cache[key] = build_program(
            S=4096, use_mask=key[0], use_bq=key[1], use_bk=key[2],
            use_bv=key[3])
    return _cache[key]


def kernel(x, mask, Wq, bq, Wk, bk, Wv, bv, Wo, bo, _results_hook=None):
    x = np.asarray(x, np.float32)
    mask = np.asarray(mask)
    B, S, _ = x.shape
    use_mask = bool((mask == 0).any())
    use_bq = bool(np.asarray(bq).any())
    use_bk = bool(np.asarray(bk).any())
    use_bv = bool(np.asarray(bv).any())
    nc = _program((use_mask, use_bq, use_bk, use_bv))

    in_maps = []
    for c in range(N_CORES):
        b, j = divmod(c, N_CORES // B)
        ds = slice(j * DL, (j + 1) * DL)
        m = {
            "xb": np.ascontiguousarray(x[b]).astype(ml_dtypes.bfloat16),
            "wq": np.ascontiguousarray(Wq[:, ds]).astype(ml_dtypes.bfloat16),
            "wk": np.ascontiguousarray(Wk[:, ds]).astype(ml_dtypes.bfloat16),
            "wv": np.ascontiguousarray(Wv[:, ds]).astype(ml_dtypes.bfloat16),
            "wo": np.ascontiguousarray(Wo[ds, :], dtype=np.float32),
        }
        if use_bq:
            m["bq"] = np.ascontiguousarray(bq[ds], dtype=np.float32)
        if use_bk:
            m["bk"] = np.ascontiguousarray(bk[ds], dtype=np.float32)
        if use_bv:
            m["bv"] = np.ascontiguousarray(bv[ds], dtype=np.float32)
        if use_mask:
            mb = np.where(np.asarray(mask[b]) == 0, -1e9, 0.0).astype(np.float32)
            m["mb"] = np.ascontiguousarray(mb.reshape(S // 128, 128).T)
        in_maps.append(m)

    res = run_bass_kernel_spmd(nc, in_maps, core_ids=list(range(N_CORES)))
    if _results_hook is not None:
        _results_hook(res)
    out = np.zeros((B, S, D_MODEL), np.float32)
    for c in range(N_CORES):
        b = c // (N_CORES // B)
        out[b] += res.results[c]["out"]
    out += np.asarray(bo, np.float32)
    return out


# revision 12
# speedup vs baseline: 1.6234x; 1.1502x over previous
"""Multi-head self-attention (B=2, S=4096, D=512, H=8, Dk=64) on 8 TRN2 cores.

Sharding: data-parallel over batch x head-parallel. Core c handles batch
c//4 and head pair (2*(c%4), 2*(c%4)+1). Each core computes Q/K/V
projections for its 128 model dims, full attention for its two heads, and
a partial output projection against its 128 rows of Wo. The host sums the
four partial outputs per batch and adds bo.

On-core layout (bf16 operands, fp32 psum accumulation):
  xT   [d, s]   bf16 via DMA-transpose of host-cast x     (rhs for Q/K, lhsT for V)
  QT/KT [128, S] bf16, head0 in partitions 0:64, head1 in 64:128
  V    [s, 128] bf16, head0 in cols 0:64, head1 in 64:128 (lhsT for ctx)
  scoresT[k, q] fp32 psum from row-packed bf16 matmul pairs (K=64/head)
  attnT = exp(scoresT/8 + mask_bias) bf16, one ACT op per [128, 1024] block
  ctxT [d, q] fp32 psum, col-packed over k blocks; denominators from
  ones-vector matmuls into psum rows 0/32; normalization via fp32 PE
  broadcast of the reciprocals.
"""

import numpy as np
import ml_dtypes
from contextlib import ExitStack

import concourse.bass as bass
import concourse.tile as tile
from concourse import bacc, mybir
from concourse.bass_utils import run_bass_kernel_spmd
from concourse.tile_rust import add_dep_helper

F32 = mybir.dt.float32
F32R = mybir.dt.float32r
BF16 = mybir.dt.bfloat16
EXP = mybir.ActivationFunctionType.Exp

D_MODEL = 512
N_HEADS = 8
D_K = 64
N_CORES = 8
DL = 128          # local model dims per core (2 heads)
Q_BLK = 512       # query block (free dim of scores matmuls)
SCALE = 1.0 / np.sqrt(D_K).item()


def build_kernel(ctx, tc, S, use_mask, use_bq, use_bk, use_bv, d):
    nc = tc.nc
    SB = S // 128    # s blocks of 128
    QB = S // Q_BLK  # query blocks of 512
    KB = S // 128    # key blocks of 128

    sp = ctx.enter_context(tc.tile_pool(name="sp", bufs=1))
    psum = ctx.enter_context(tc.tile_pool(name="psum", bufs=1, space="PSUM"))
    # psum budget (8 banks): scores 2x[128,1024]=4, ctx 2x[128,512]=2,
    # den 2x[<=1 bank]=2. All other matmul outputs share the ctx/den tags.

    # ---- constants ----
    ones_f = sp.tile([128, 1], F32, tag="ones_f")
    nc.vector.memset(ones_f, 1.0)
    ones_col = sp.tile([128, 1], BF16, tag="ones_col")  # lhsT of denominator mms
    nc.vector.tensor_copy(ones_col, ones_f)
    ones_rep = sp.tile([33, 128], F32, tag="ones_rep")  # lhsT of broadcast mms
    nc.vector.memset(ones_rep, 1.0)

    # ---- phase 1: DMA-transpose x (bf16) into xT [128, 4, S] ----
    # The xbar-transpose DMAs must not overlap regular DMAs (S2M xbar-mode
    # transition hazard), so they go first and every other DMA waits on them.
    xt = sp.tile([128, 4, S], BF16, tag="xt")
    t_insts = []
    for t in range(4):
        t_insts.append(nc.sync.dma_start_transpose(
            xt[:, t, :], d["xb"].ap()[:, t * 128:(t + 1) * 128]))

    def dma_after_transpose(out_ap, in_ap):
        ins = nc.sync.dma_start(out_ap, in_ap)
        for t_inst in t_insts:
            add_dep_helper(ins.ins, t_inst.ins, reason="xbar-mode serialize")
        return ins

    # ---- weights ----
    wq_sb = sp.tile([128, 4, 128], BF16, tag="wq")
    dma_after_transpose(wq_sb, d["wq"].ap().rearrange("(t p) d -> p t d", p=128))
    wk_sb = sp.tile([128, 4, 128], BF16, tag="wk")
    dma_after_transpose(wk_sb, d["wk"].ap().rearrange("(t p) d -> p t d", p=128))
    wv_sb = sp.tile([128, 4, 128], BF16, tag="wv")
    dma_after_transpose(wv_sb, d["wv"].ap().rearrange("(t p) d -> p t d", p=128))
    wo_sb = sp.tile([128, 512], F32R, tag="wo")
    dma_after_transpose(wo_sb, d["wo"].ap())
    if use_bq:
        bq_sb = sp.tile([128, 1], F32, tag="bq")
        dma_after_transpose(bq_sb, d["bq"].ap()[:, None])
    if use_bk:
        bk_sb = sp.tile([128, 1], F32, tag="bk")
        dma_after_transpose(bk_sb, d["bk"].ap()[:, None])
    if use_bv:
        bv_sb = sp.tile([1, 128], F32, tag="bv")
        dma_after_transpose(bv_sb, d["bv"].ap()[None, :])
        ones_row = sp.tile([1, 128], F32, tag="ones_row")
        nc.vector.memset(ones_row, 1.0)
    if use_mask:
        mb_sb = sp.tile([128, KB], F32, tag="mb")
        dma_after_transpose(mb_sb, d["mb"].ap())

    # ---- phase 2: projections ----
    qt = sp.tile([128, S], BF16, tag="qt")
    kt = sp.tile([128, S], BF16, tag="kt")
    for dst, w_sb, b_sb in (
        (qt, wq_sb, bq_sb if use_bq else None),
        (kt, wk_sb, bk_sb if use_bk else None),
    ):
        for sc in range(S // 512):
            pp = psum.tile([128, 512], F32, tag="ctx", bufs=2)
            for t in range(4):
                nc.tensor.matmul(
                    pp, w_sb[:, t, :], xt[:, t, sc * 512:(sc + 1) * 512],
                    start=(t == 0), stop=(t == 3))
            out = dst[:, sc * 512:(sc + 1) * 512]
            if b_sb is not None:
                nc.vector.tensor_scalar_add(out, pp, b_sb[:, 0:1])
            else:
                nc.vector.tensor_copy(out, pp)

    v_all = sp.tile([128, SB, 128], BF16, tag="v")
    for sb in range(SB):
        pv = psum.tile([128, 128], F32, tag="den", bufs=2)
        for t in range(4):
            nc.tensor.matmul(
                pv, xt[:, t, sb * 128:(sb + 1) * 128], wv_sb[:, t, :],
                start=(t == 0), stop=(t == 3 and not use_bv))
        if use_bv:
            nc.tensor.matmul(pv, ones_row[0:1, :], bv_sb[0:1, :],
                             start=False, stop=True)
        nc.vector.tensor_copy(v_all[:, sb, :], pv)

    # ---- phase 3: attention ----
    ctxn = sp.tile([128, S], F32R, tag="ctxn")
    pending_tail = None
    for qb in range(QB):
        qs = slice(qb * Q_BLK, (qb + 1) * Q_BLK)
        pc = psum.tile([128, 512], F32, tag="ctx", bufs=2)
        pd = psum.tile([33, 512], F32, tag="den", bufs=2)

        def scores_block(kb):
            # one query-block column of scores for both heads + its exp
            ks = slice(kb * 128, (kb + 1) * 128)
            ps = psum.tile([128, 1024], F32, tag="scores", bufs=2, name="ps")
            nc.tensor.matmul(ps[:, 0:512], kt[0:64, ks], qt[0:64, qs])
            nc.tensor.matmul(ps[:, 512:1024], kt[64:128, ks], qt[64:128, qs])
            attn = sp.tile([128, 1024], BF16, tag="attn", bufs=3, name="attn")
            nc.scalar.activation(
                attn, ps, EXP, scale=SCALE,
                bias=mb_sb[:, kb:kb + 1] if use_mask else 0.0)
            return attn

        # Software-pipelined: scores/exp for kb+1 are emitted before the
        # ctx/den matmuls of kb, so the PE streams scores(kb+1) while the
        # ACT engine computes exp(kb) — the serial exp->ctx->scores->exp
        # chain would otherwise set the loop period.
        attn = scores_block(0)
        for kb in range(KB):
            attn_next = scores_block(kb + 1) if kb + 1 < KB else None
            if kb == 3 and pending_tail is not None:
                pending_tail()
                pending_tail = None
            first, last = kb == 0, kb == KB - 1
            nc.tensor.matmul(pc[0:64, :], v_all[:, kb, 0:64],
                             attn[:, 0:512], start=first, stop=last,
                             skip_group_check=True)
            nc.tensor.matmul(pc[64:128, :], v_all[:, kb, 64:128],
                             attn[:, 512:1024], start=first, stop=last,
                             skip_group_check=True)
            nc.tensor.matmul(pd[0:1, :], ones_col[:, 0:1],
                             attn[:, 0:512], start=first, stop=last,
                             skip_group_check=True)
            nc.tensor.matmul(pd[32:33, :], ones_col[:, 0:1],
                             attn[:, 512:1024], start=first, stop=last,
                             skip_group_check=True)
            attn = attn_next

        # Denominator extraction + reciprocals start immediately (DVE is
        # idle during the matmul loop)...
        den_sb = sp.tile([33, 512], F32, tag="den_sb", bufs=2)
        nc.vector.tensor_copy(den_sb[0:1, :], pd[0:1, :])
        nc.vector.tensor_copy(den_sb[32:33, :], pd[32:33, :])
        rcp = sp.tile([33, 512], F32, tag="rcp", bufs=2)
        nc.vector.reciprocal(rcp[0:1, :], den_sb[0:1, :])
        nc.vector.reciprocal(rcp[32:33, :], den_sb[32:33, :])

        # ...but the PE part of the tail (broadcast matmuls + output
        # projection) is deferred by one query block, so the PE never
        # stalls on the reciprocal chain — that stall re-throttles the
        # HAM clock to K=4/8 and halves matmul throughput.
        def tail(qb=qb, qs=qs, pc=pc, rcp=rcp):
            pr0 = psum.tile([128, 512], F32, tag="den", bufs=2, name="pr0")
            nc.tensor.matmul(pr0, ones_rep[0:1, :], rcp[0:1, :])
            rep0 = sp.tile([128, 512], F32, tag="rep", bufs=2, name="rep0")
            nc.vector.tensor_copy(rep0, pr0)
            pr1 = psum.tile([128, 512], F32, tag="den", bufs=2, name="pr1")
            nc.tensor.matmul(pr1, ones_rep[32:33, :], rcp[32:33, :])
            rep1 = sp.tile([128, 512], F32, tag="rep", bufs=2, name="rep1")
            nc.vector.tensor_copy(rep1, pr1)
            nc.vector.tensor_mul(ctxn[0:64, qs], pc[0:64, :], rep0[0:64, :])
            nc.vector.tensor_mul(ctxn[64:128, qs], pc[64:128, :], rep1[64:128, :])
            for i in range(Q_BLK // 128):
                sb = qb * (Q_BLK // 128) + i
                po = psum.tile([128, 512], F32, tag="ctx", bufs=2, name="po")
                nc.tensor.matmul(po, ctxn[:, sb * 128:(sb + 1) * 128], wo_sb)
                ob = sp.tile([128, 512], F32, tag="ob", bufs=3, name="ob")
                nc.vector.tensor_copy(ob, po)
                nc.sync.dma_start(d["out"].ap()[sb * 128:(sb + 1) * 128, :], ob)

        pending_tail = tail

    pending_tail()


def build_program(S=4096, use_mask=False, use_bq=False, use_bk=False,
                  use_bv=False, enable_asserts=False):
    nc = bacc.Bacc("TRN2", target_bir_lowering=False, debug=False,
                   enable_asserts=enable_asserts, num_devices=N_CORES,
                   name="mha")
    d = {
        "xb": nc.dram_tensor("xb", [S, D_MODEL], BF16, kind="ExternalInput"),
        "wq": nc.dram_tensor("wq", [D_MODEL, DL], BF16, kind="ExternalInput"),
        "wk": nc.dram_tensor("wk", [D_MODEL, DL], BF16, kind="ExternalInput"),
        "wv": nc.dram_tensor("wv", [D_MODEL, DL], BF16, kind="ExternalInput"),
        "wo": nc.dram_tensor("wo", [DL, D_MODEL], F32R, kind="ExternalInput"),
        "out": nc.dram_tensor("out", [S, D_MODEL], F32, kind="ExternalOutput"),
    }
    if use_bq:
        d["bq"] = nc.dram_tensor("bq", [DL], F32, kind="ExternalInput")
    if use_bk:
        d["bk"] = nc.dram_tensor("bk", [DL], F32, kind="ExternalInput")
    if use_bv:
        d["bv"] = nc.dram_tensor("bv", [DL], F32, kind="ExternalInput")
    if use_mask:
        d["mb"] = nc.dram_tensor("mb", [128, S // 128], F32,
                                 kind="ExternalInput")
    with tile.TileContext(nc) as tc:
        with ExitStack() as ctx:
            build_kernel(ctx, tc, S, use_mask, use_bq, use_bk, use_bv, d)
    nc.compile()
    return nc


_cache = {}


def _program(key):
    if key not in _cache:
        _cache[key] = build_program(
            S=4096, use_mask=key[0], use_bq=key[1], use_bk=key[2],
            use_bv=key[3])
    return _cache[key]


def kernel(x, mask, Wq, bq, Wk, bk, Wv, bv, Wo, bo, _results_hook=None):
    x = np.asarray(x, np.float32)
    mask = np.asarray(mask)
    B, S, _ = x.shape
    use_mask = bool((mask == 0).any())
    use_bq = bool(np.asarray(bq).any())
    use_bk = bool(np.asarray(bk).any())
    use_bv = bool(np.asarray(bv).any())
    nc = _program((use_mask, use_bq, use_bk, use_bv))

    in_maps = []
    for c in range(N_CORES):
        b, j = divmod(c, N_CORES // B)
        ds = slice(j * DL, (j + 1) * DL)
        m = {
            "xb": np.ascontiguousarray(x[b]).astype(ml_dtypes.bfloat16),
            "wq": np.ascontiguousarray(Wq[:, ds]).astype(ml_dtypes.bfloat16),
            "wk": np.ascontiguousarray(Wk[:, ds]).astype(ml_dtypes.bfloat16),
            "wv": np.ascontiguousarray(Wv[:, ds]).astype(ml_dtypes.bfloat16),
            "wo": np.ascontiguousarray(Wo[ds, :], dtype=np.float32),
        }
        if use_bq:
            m["bq"] = np.ascontiguousarray(bq[ds], dtype=np.float32)
        if use_bk:
            m["bk"] = np.ascontiguousarray(bk[ds], dtype=np.float32)
        if use_bv:
            m["bv"] = np.ascontiguousarray(bv[ds], dtype=np.float32)
        if use_mask:
            mb = np.where(np.asarray(mask[b]) == 0, -1e9, 0.0).astype(np.float32)
            m["mb"] = np.ascontiguousarray(mb.reshape(S // 128, 128).T)
        in_maps.append(m)

    res = run_bass_kernel_spmd(nc, in_maps, core_ids=list(range(N_CORES)))
    if _results_hook is not None:
        _results_hook(res)
    out = np.zeros((B, S, D_MODEL), np.float32)
    for c in range(N_CORES):
        b = c // (N_CORES // B)
        out[b] += res.results[c]["out"]
    out += np.asarray(bo, np.float32)
    return out


# revision 14
# speedup vs baseline: 1.9130x; 1.1784x over previous
"""Multi-head self-attention (B=2, S=4096, D=512, H=8, Dk=64) on 8 TRN2 cores.

Sharding: data-parallel over batch x head-parallel. Core c handles batch
c//4 and head pair (2*(c%4), 2*(c%4)+1). Each core computes Q/K/V
projections for its 128 model dims, full attention for its two heads, and
a partial output projection against its 128 rows of Wo. The host sums the
four partial outputs per batch and adds bo.

On-core layout (bf16 operands, fp32 psum accumulation):
  xT   [d, s]   bf16 via DMA-transpose of host-cast x     (rhs for Q/K, lhsT for V)
  QT/KT [128, S] bf16, head0 in partitions 0:64, head1 in 64:128
  V    [s, 128] bf16, head0 in cols 0:64, head1 in 64:128 (lhsT for ctx)
  scoresT[k, q] fp32 psum from row-packed bf16 matmul pairs (K=64/head)
  attnT = exp(scoresT/8 + mask_bias) bf16, one ACT op per [128, 1024] block
  ctxT [d, q] fp32 psum, col-packed over k blocks; denominators from
  ones-vector matmuls into psum rows 0/32; normalization via fp32 PE
  broadcast of the reciprocals.
"""

import numpy as np
import ml_dtypes
from contextlib import ExitStack

import concourse.bass as bass
import concourse.tile as tile
from concourse import bacc, mybir
from concourse.bass_utils import run_bass_kernel_spmd
from concourse.tile_rust import add_dep_helper

F32 = mybir.dt.float32
F32R = mybir.dt.float32r
BF16 = mybir.dt.bfloat16
EXP = mybir.ActivationFunctionType.Exp

D_MODEL = 512
N_HEADS = 8
D_K = 64
N_CORES = 8
DL = 128          # local model dims per core (2 heads)
Q_BLK = 512       # query block (free dim of scores matmuls)
SCALE = 1.0 / np.sqrt(D_K).item()


def build_kernel(ctx, tc, S, use_mask, use_bq, use_bk, use_bv, d):
    nc = tc.nc
    SB = S // 128    # s blocks of 128
    QB = S // Q_BLK  # query blocks of 512
    KB = S // 128    # key blocks of 128

    sp = ctx.enter_context(tc.tile_pool(name="sp", bufs=1))
    psum = ctx.enter_context(tc.tile_pool(name="psum", bufs=1, space="PSUM"))
    # psum budget (8 banks): scores 2x[128,1024]=4, ctx 2x[128,512]=2,
    # den 2x[<=1 bank]=2. All other matmul outputs share the ctx/den tags.

    # ---- constants ----
    ones_f = sp.tile([128, 1], F32, tag="ones_f")
    nc.vector.memset(ones_f, 1.0)
    ones_col = sp.tile([128, 1], BF16, tag="ones_col")  # lhsT of denominator mms
    nc.vector.tensor_copy(ones_col, ones_f)
    ones_rep_f = sp.tile([33, 128], F32, tag="ones_rep_f")
    nc.vector.memset(ones_rep_f, 1.0)
    ones_rep = sp.tile([33, 128], F32R, tag="ones_rep")  # lhsT of broadcast mms
    nc.vector.tensor_copy(ones_rep, ones_rep_f)

    # ---- phase 1: DMA-transpose x (bf16) into xT [128, 4, S] ----
    # The xbar-transpose DMAs must not overlap regular DMAs (S2M xbar-mode
    # transition hazard), so they go first and every other DMA waits on them.
    xt = sp.tile([128, 4, S], BF16, tag="xt")
    t_insts = []
    for t in range(4):
        t_insts.append(nc.sync.dma_start_transpose(
            xt[:, t, :], d["xb"].ap()[:, t * 128:(t + 1) * 128]))

    def dma_after_transpose(out_ap, in_ap):
        ins = nc.sync.dma_start(out_ap, in_ap)
        for t_inst in t_insts:
            add_dep_helper(ins.ins, t_inst.ins, reason="xbar-mode serialize")
        return ins

    # ---- weights ----
    wq_sb = sp.tile([128, 4, 128], BF16, tag="wq")
    dma_after_transpose(wq_sb, d["wq"].ap().rearrange("(t p) d -> p t d", p=128))
    wk_sb = sp.tile([128, 4, 128], BF16, tag="wk")
    dma_after_transpose(wk_sb, d["wk"].ap().rearrange("(t p) d -> p t d", p=128))
    wv_sb = sp.tile([128, 4, 128], BF16, tag="wv")
    dma_after_transpose(wv_sb, d["wv"].ap().rearrange("(t p) d -> p t d", p=128))
    wo_sb = sp.tile([128, 512], F32R, tag="wo")
    dma_after_transpose(wo_sb, d["wo"].ap())
    if use_bq:
        bq_sb = sp.tile([128, 1], F32, tag="bq")
        dma_after_transpose(bq_sb, d["bq"].ap()[:, None])
    if use_bk:
        bk_sb = sp.tile([128, 1], F32, tag="bk")
        dma_after_transpose(bk_sb, d["bk"].ap()[:, None])
    if use_bv:
        bv_sb = sp.tile([1, 128], F32, tag="bv")
        dma_after_transpose(bv_sb, d["bv"].ap()[None, :])
        ones_row = sp.tile([1, 128], F32, tag="ones_row")
        nc.vector.memset(ones_row, 1.0)
    if use_mask:
        mb_sb = sp.tile([128, KB], F32, tag="mb")
        dma_after_transpose(mb_sb, d["mb"].ap())

    # ---- phase 2: projections ----
    qt = sp.tile([128, S], BF16, tag="qt")
    kt = sp.tile([128, S], BF16, tag="kt")
    for dst, w_sb, b_sb in (
        (qt, wq_sb, bq_sb if use_bq else None),
        (kt, wk_sb, bk_sb if use_bk else None),
    ):
        for sc in range(S // 512):
            pp = psum.tile([128, 512], F32, tag="ctx", bufs=2)
            for t in range(4):
                nc.tensor.matmul(
                    pp, w_sb[:, t, :], xt[:, t, sc * 512:(sc + 1) * 512],
                    start=(t == 0), stop=(t == 3))
            out = dst[:, sc * 512:(sc + 1) * 512]
            if b_sb is not None:
                nc.vector.tensor_scalar_add(out, pp, b_sb[:, 0:1])
            else:
                nc.vector.tensor_copy(out, pp)

    v_all = sp.tile([128, SB, 128], BF16, tag="v")
    for sb in range(SB):
        pv = psum.tile([128, 128], F32, tag="den", bufs=2)
        for t in range(4):
            nc.tensor.matmul(
                pv, xt[:, t, sb * 128:(sb + 1) * 128], wv_sb[:, t, :],
                start=(t == 0), stop=(t == 3 and not use_bv))
        if use_bv:
            nc.tensor.matmul(pv, ones_row[0:1, :], bv_sb[0:1, :],
                             start=False, stop=True)
        nc.vector.tensor_copy(v_all[:, sb, :], pv)

    # ---- phase 3: attention ----
    ctxn = sp.tile([128, S], F32R, tag="ctxn")
    pending_tail = None
    for qb in range(QB):
        qs = slice(qb * Q_BLK, (qb + 1) * Q_BLK)
        pc = psum.tile([128, 512], F32, tag="ctx", bufs=2)
        pd = psum.tile([33, 512], F32, tag="den", bufs=2)

        def scores_block(kb):
            # one query-block column of scores for both heads + its exp
            ks = slice(kb * 128, (kb + 1) * 128)
            ps = psum.tile([128, 1024], F32, tag="scores", bufs=2, name="ps")
            nc.tensor.matmul(ps[:, 0:512], kt[0:64, ks], qt[0:64, qs])
            nc.tensor.matmul(ps[:, 512:1024], kt[64:128, ks], qt[64:128, qs])
            attn = sp.tile([128, 1024], BF16, tag="attn", bufs=3, name="attn")
            nc.scalar.activation(
                attn, ps, EXP, scale=SCALE,
                bias=mb_sb[:, kb:kb + 1] if use_mask else 0.0)
            return attn

        # Software-pipelined: scores/exp for kb+1 are emitted before the
        # ctx/den matmuls of kb, so the PE streams scores(kb+1) while the
        # ACT engine computes exp(kb) — the serial exp->ctx->scores->exp
        # chain would otherwise set the loop period.
        attn = scores_block(0)
        anchor = None
        for kb in range(KB):
            attn_next = scores_block(kb + 1) if kb + 1 < KB else None
            if kb == 8 and pending_tail is not None:
                pending_tail(anchor)
                pending_tail = None
            first, last = kb == 0, kb == KB - 1
            m = nc.tensor.matmul(pc[0:64, :], v_all[:, kb, 0:64],
                                 attn[:, 0:512], start=first, stop=last,
                                 skip_group_check=True)
            if kb == 7:
                anchor = m
            nc.tensor.matmul(pc[64:128, :], v_all[:, kb, 64:128],
                             attn[:, 512:1024], start=first, stop=last,
                             skip_group_check=True)
            nc.tensor.matmul(pd[0:1, :], ones_col[:, 0:1],
                             attn[:, 0:512], start=first, stop=last,
                             skip_group_check=True)
            nc.tensor.matmul(pd[32:33, :], ones_col[:, 0:1],
                             attn[:, 512:1024], start=first, stop=last,
                             skip_group_check=True)
            attn = attn_next

        # Denominator extraction + reciprocals start immediately (DVE is
        # idle during the matmul loop)...
        den_sb = sp.tile([33, 512], F32, tag="den_sb", bufs=2)
        nc.vector.tensor_copy(den_sb[0:1, :], pd[0:1, :])
        nc.vector.tensor_copy(den_sb[32:33, :], pd[32:33, :])
        rcp = sp.tile([33, 512], F32R, tag="rcp", bufs=2)
        with nc.allow_low_precision(reason="f32r-rounded reciprocal feeds the fp32r broadcast matmul"):
            nc.vector.reciprocal(rcp[0:1, :], den_sb[0:1, :])
            nc.vector.reciprocal(rcp[32:33, :], den_sb[32:33, :])

        # ...but the PE part of the tail (broadcast matmuls + output
        # projection) is deferred by one query block, so the PE never
        # stalls on the reciprocal chain — that stall re-throttles the
        # HAM clock to K=4/8 and halves matmul throughput.
        def tail(anchor, qb=qb, qs=qs, pc=pc, rcp=rcp):
            pr0 = psum.tile([128, 512], F32, tag="den", bufs=2, name="pr0")
            m0 = nc.tensor.matmul(pr0, ones_rep[0:1, :], rcp[0:1, :])
            rep0 = sp.tile([128, 512], F32, tag="rep", bufs=2, name="rep0")
            nc.vector.tensor_copy(rep0, pr0)
            pr1 = psum.tile([128, 512], F32, tag="den", bufs=2, name="pr1")
            m1 = nc.tensor.matmul(pr1, ones_rep[32:33, :], rcp[32:33, :])
            rep1 = sp.tile([128, 512], F32, tag="rep", bufs=2, name="rep1")
            nc.vector.tensor_copy(rep1, pr1)
            if anchor is not None:
                add_dep_helper(m0.ins, anchor.ins, reason="defer tail mm")
                add_dep_helper(m1.ins, anchor.ins, reason="defer tail mm")
            nc.vector.tensor_mul(ctxn[0:64, qs], pc[0:64, :], rep0[0:64, :])
            nc.vector.tensor_mul(ctxn[64:128, qs], pc[64:128, :], rep1[64:128, :])
            for i in range(Q_BLK // 128):
                sb = qb * (Q_BLK // 128) + i
                po = psum.tile([128, 512], F32, tag="ctx", bufs=2, name="po")
                nc.tensor.matmul(po, ctxn[:, sb * 128:(sb + 1) * 128], wo_sb)
                ob = sp.tile([128, 512], F32, tag="ob", bufs=3, name="ob")
                nc.vector.tensor_copy(ob, po)
                nc.sync.dma_start(d["out"].ap()[sb * 128:(sb + 1) * 128, :], ob)

        pending_tail = tail

    pending_tail(None)


def build_program(S=4096, use_mask=False, use_bq=False, use_bk=False,
                  use_bv=False, enable_asserts=False):
    nc = bacc.Bacc("TRN2", target_bir_lowering=False, debug=False,
                   enable_asserts=enable_asserts, num_devices=N_CORES,
                   name="mha")
    d = {
        "xb": nc.dram_tensor("xb", [S, D_MODEL], BF16, kind="ExternalInput"),
        "wq": nc.dram_tensor("wq", [D_MODEL, DL], BF16, kind="ExternalInput"),
        "wk": nc.dram_tensor("wk", [D_MODEL, DL], BF16, kind="ExternalInput"),
        "wv": nc.dram_tensor("wv", [D_MODEL, DL], BF16, kind="ExternalInput"),
        "wo": nc.dram_tensor("wo", [DL, D_MODEL], F32R, kind="ExternalInput"),
        "out": nc.dram_tensor("out", [S, D_MODEL], F32, kind="ExternalOutput"),
    }
    if use_bq:
        d["bq"] = nc.dram_tensor("bq", [DL], F32, kind="ExternalInput")
    if use_bk:
        d["bk"] = nc.dram_tensor("bk", [DL], F32, kind="ExternalInput")
    if use_bv:
        d["bv"] = nc.dram_tensor("bv", [DL], F32, kind="ExternalInput")
    if use_mask:
        d["mb"] = nc.dram_tensor("mb", [128, S // 128], F32,
                                 kind="ExternalInput")
    with tile.TileContext(nc) as tc:
        with ExitStack() as ctx:
            build_kernel(ctx, tc, S, use_mask, use_bq, use_bk, use_bv, d)
    nc.compile()
    return nc


_cache = {}


def _program(key):
    if key not in _cache:
        _cache[key] = build_program(
            S=4096, use_mask=key[0], use_bq=key[1], use_bk=key[2],
            use_bv=key[3])
    return _cache[key]


def kernel(x, mask, Wq, bq, Wk, bk, Wv, bv, Wo, bo, _results_hook=None):
    x = np.asarray(x, np.float32)
    mask = np.asarray(mask)
    B, S, _ = x.shape
    use_mask = bool((mask == 0).any())
    use_bq = bool(np.asarray(bq).any())
    use_bk = bool(np.asarray(bk).any())
    use_bv = bool(np.asarray(bv).any())
    nc = _program((use_mask, use_bq, use_bk, use_bv))

    in_maps = []
    for c in range(N_CORES):
        b, j = divmod(c, N_CORES // B)
        ds = slice(j * DL, (j + 1) * DL)
        m = {
            "xb": np.ascontiguousarray(x[b]).astype(ml_dtypes.bfloat16),
            "wq": np.ascontiguousarray(Wq[:, ds]).astype(ml_dtypes.bfloat16),
            "wk": np.ascontiguousarray(Wk[:, ds]).astype(ml_dtypes.bfloat16),
            "wv": np.ascontiguousarray(Wv[:, ds]).astype(ml_dtypes.bfloat16),
            "wo": np.ascontiguousarray(Wo[ds, :], dtype=np.float32),
        }
        if use_bq:
            m["bq"] = np.ascontiguousarray(bq[ds], dtype=np.float32)
        if use_bk:
            m["bk"] = np.ascontiguousarray(bk[ds], dtype=np.float32)
        if use_bv:
            m["bv"] = np.ascontiguousarray(bv[ds], dtype=np.float32)
        if use_mask:
            mb = np.where(np.asarray(mask[b]) == 0, -1e9, 0.0).astype(np.float32)
            m["mb"] = np.ascontiguousarray(mb.reshape(S // 128, 128).T)
        in_maps.append(m)

    res = run_bass_kernel_spmd(nc, in_maps, core_ids=list(range(N_CORES)))
    if _results_hook is not None:
        _results_hook(res)
    out = np.zeros((B, S, D_MODEL), np.float32)
    for c in range(N_CORES):
        b = c // (N_CORES // B)
        out[b] += res.results[c]["out"]
    out += np.asarray(bo, np.float32)
    return out
